# revision 54
# baseline (speedup 1.0000x reference)
"""Trainium2 Bass kernel for nn_BertCounterFactCrossOpitionCompetitionTransformer.

v2: packed/slot-max layouts + fp8-e4m3 DoubleRow projections & scores.

Strategy (data-parallel over batch, 4 slot-batches per core on 8 cores):

Batches are sorted by false-segment length and dealt round-robin so all
cores carry near-equal work and slot-wise max sizes (shared by the SPMD
program) are tight.  Per core and slot i (sizes f<=fhat[i], o<=ohat[i]):

  C_t   = [Wq;bq][Wk;bk]^T * inv                  (host)
  G_t^T = (C_t*SC)^T xf_aug^T / (SC/SG)           (PE fp8 DoubleRow, packed N)
  S_t   = G_t xo_aug^T (*SG) (+mask via K=1 bf16 matmul)   (PE fp8 DR)
  P_t   = exp(S/SG) with rowsum Z (ACT accum); rep adds tanh(con) first
  gate  = softmax over anom (DVE dots + tiny PE sums)
  u_t   = P_t^T (eg/Z)                            (PE, batch-local chunks)
  vecs  = [eg | u_rep | u_sup] batched against xf_r/xo_r   (PE, 12 rows)
  fusedT via one DMA xbar transpose; h = relu(W1^T fused) via 4
  concurrent PE column-groups; hT via PE transposes; y = h W2 (+mu col),
  layernorm with DVE square-accum and preloaded Rsqrt.

fp8 is used ONLY where averaging washes quantization out (proj+scores,
host-verified 4.3e-3 vs bf16 3.3e-3); gate/vec/fuse stay bf16.
"""

import numpy as np
import ml_dtypes

B, L, D = 32, 512, 768
NCORES = 8
BPC = B // NCORES
NEGM = -30000.0
INV = 1.0 / np.sqrt(D)
P = 128
BF16 = ml_dtypes.bfloat16
F8 = ml_dtypes.float8_e4m3

SC = 2048.0    # host scale on C; G and scores carry it on device too
NKF = 19       # k-tiles over 2304+1 fuse contraction
HW = [(0, 256), (256, 256), (512, 128), (640, 128)]  # h col-groups


def _segment_masks(x_ids, pad_idx, sep_idx):
    sep_mask = x_ids == sep_idx
    has_sep = sep_mask.any(axis=1)
    idxs = np.argmax(sep_mask.astype(np.int32), axis=1)
    valid_mask = x_ids != pad_idx
    valid_len = valid_mask.sum(axis=1)
    fallback = np.clip(valid_len // 2, 1, max(1, L - 2))
    sep_pos = np.where(has_sep, idxs, fallback)
    pos = np.arange(L)[None, :]
    false_mask = (pos < sep_pos[:, None]) & valid_mask
    option_mask = (pos > sep_pos[:, None]) & valid_mask
    return false_mask, option_mask


def _ceil(a, m):
    return -(-a // m) * m


class Sizes:
    """Slot-wise shared sizes derived from the actual input masks."""

    def __init__(self, fhat, ohat, trivial_ln):
        self.fhat = tuple(int(v) for v in fhat)
        self.ohat = tuple(int(v) for v in ohat)
        self.trivial_ln = bool(trivial_ln)
        self.NFC = [-(-f // P) for f in self.fhat]
        self.NOC = [-(-o // P) for o in self.ohat]
        self.FOFF = np.concatenate([[0], np.cumsum(self.fhat)]).astype(int)
        self.JF = np.concatenate([[0], np.cumsum(self.NFC)]).astype(int)
        self.JO = np.concatenate([[0], np.cumsum(self.NOC)]).astype(int)
        self.NQF = int(self.FOFF[-1])
        self.NQF16 = _ceil(self.NQF, 16)
        self.NJF = int(self.JF[-1])
        self.NJO = int(self.JO[-1])
        self.NQO = P * self.NJO

    def key(self):
        return (self.fhat, self.ohat, self.trivial_ln)


def _chunks(n, step=512):
    out = []
    c = 0
    while c < n:
        out.append((c, min(step, n - c)))
        c += step
    return out


def _build_program(sz: Sizes):
    import concourse.bacc as bacc
    import concourse.mybir as mybir
    import concourse.tile as tile

    fp32 = mybir.dt.float32
    bf16 = mybir.dt.bfloat16
    f8e4 = mybir.dt.float8e4
    AF = mybir.ActivationFunctionType
    DR = mybir.MatmulPerfMode.DoubleRow
    AX = mybir.AxisListType.X
    ALU = mybir.AluOpType

    NQF, NQF16, NQO = sz.NQF, sz.NQF16, sz.NQO
    NJF, NJO = sz.NJF, sz.NJO
    fhat, ohat, NFC, NOC = sz.fhat, sz.ohat, sz.NFC, sz.NOC
    FOFF, JF, JO = sz.FOFF, sz.JF, sz.JO

    nc = bacc.Bacc("TRN2", target_bir_lowering=False, debug=False)

    di = {}
    def dram_in(name, shape, dt):
        di[name] = nc.dram_tensor(name, list(shape), dt, kind="ExternalInput")
        return di[name]

    dram_in("xfT8", (P, 8, NQF16), f8e4)
    dram_in("xoT8", (P, 6, NQO), f8e4)
    dram_in("wc8_con", (P, 8, 784), f8e4)
    for t in ("sup", "rep"):
        dram_in(f"wc8_{t}", (P, 8, D), f8e4)
    dram_in("xf_r", (P, NJF, D), bf16)
    dram_in("xo_r", (P, NJO, D), bf16)
    dram_in("mo_row", (1, NQO), bf16)
    dram_in("wa_bc", (P, D), bf16)
    dram_in("mask_col", (P, NJF, 1), bf16)
    dram_in("w1", (P, NKF, D), bf16)
    dram_in("w2", (P, 7, D + 1), bf16)
    dram_in("eye", (P, 68), bf16)
    dram_in("eyesg", (P, P), bf16)
    if not sz.trivial_ln:
        dram_in("lng", (BPC, D), fp32)
        dram_in("lnb", (BPC, D), fp32)
    y_out = nc.dram_tensor("y", [BPC, D], fp32, kind="ExternalOutput")

    with tile.TileContext(nc) as tc:
        with (
            tc.tile_pool(name="const", bufs=1) as const,
            tc.tile_pool(name="xin", bufs=1) as xin,
            tc.tile_pool(name="wcp", bufs=2) as wcp,
            tc.tile_pool(name="wfuse", bufs=1) as wfuse,
            tc.tile_pool(name="gt", bufs=2) as gtp,
            tc.tile_pool(name="soft", bufs=4) as soft,
            tc.tile_pool(name="stats", bufs=1) as stats,
            tc.tile_pool(name="psum_big", bufs=3, space="PSUM") as pp_big,
            tc.tile_pool(name="psum_s", bufs=4, space="PSUM") as pp_s,
            tc.tile_pool(name="psum_sm", bufs=1, space="PSUM") as pp_sm,
        ):
            def load(name, shape, dt, pool=xin, tag=None, name_=None,
                     split=None):
                t_ = pool.tile(list(shape), dt, tag=tag or name,
                               name=name_ or f"sb_{name}")
                if split:
                    for k0, kn in split:
                        nc.sync.dma_start(out=t_[:, k0:k0 + kn, :],
                                          in_=di[name][:, k0:k0 + kn, :])
                else:
                    nc.sync.dma_start(out=t_[:], in_=di[name][:])
                return t_

            # ---- consts ----
            ones_row = const.tile([1, P], bf16, tag="ones_row")
            nc.vector.memset(ones_row[:], 1.0)
            ones_col = const.tile([P, 1], bf16, tag="ones_col")
            nc.vector.memset(ones_col[:], 1.0)
            zbias = const.tile([P, 1], fp32, tag="zbias")
            nc.vector.memset(zbias[:], 0.0)
            eps_t = const.tile([P, 1], fp32, tag="eps")
            nc.vector.memset(eps_t[:], 1e-5)
            junk1 = const.tile([1, 1], fp32, tag="junk1")
            maskl = const.tile([1, 3], bf16, tag="maskl")
            nc.vector.memset(maskl[0:1, 0:1], 1.0)
            nc.vector.memset(maskl[0:1, 1:3], 0.0)

            # ---- PE warmup: release the HAM clock gate during DMA wait ----
            scr = const.tile([P, 512], bf16, tag="scr")
            nc.vector.memset(scr[:], 1.0)
            ps_wu = pp_sm.tile([P, 512], fp32, tag="sm", name="ps_wu")
            for r in range(10):
                nc.tensor.matmul(ps_wu[:], scr[:, 0:P], scr[:],
                                 start=(r == 0), stop=(r == 9))
            # ---- input DMAs in priority order, split across the two
            # HWDGE rings (sync + scalar) to halve issue serialization ----
            def load2(name, shape, dt, eng, pool=xin, tag=None, name_=None,
                      split=None):
                t_ = pool.tile(list(shape), dt, tag=tag or name,
                               name=name_ or f"sb_{name}")
                if split:
                    for k0, kn in split:
                        eng.dma_start(out=t_[:, k0:k0 + kn, :],
                                      in_=di[name][:, k0:k0 + kn, :])
                else:
                    eng.dma_start(out=t_[:], in_=di[name][:])
                return t_

            wc = {"con": wcp.tile([P, 8, 784], f8e4, tag="wc", name="wc_con")}
            xfT8 = xin.tile([P, 8, NQF16], f8e4, tag="xfT8", name="sb_xfT8")
            for k in range(4):
                nc.sync.dma_start(out=wc["con"][:, 2 * k:2 * k + 2, :],
                                  in_=di["wc8_con"][:, 2 * k:2 * k + 2, :])
                nc.sync.dma_start(out=xfT8[:, 2 * k:2 * k + 2, :],
                                  in_=di["xfT8"][:, 2 * k:2 * k + 2, :])
            xoT8 = load2("xoT8", (P, 6, NQO), f8e4, nc.sync,
                         split=[(2 * k, 2) for k in range(3)])
            eye = load2("eye", (P, 68), bf16, nc.sync, pool=wfuse,
                        tag="eye")
            eyesg = load2("eyesg", (P, P), bf16, nc.sync, pool=wfuse,
                          tag="eyesg")
            mo_row = load2("mo_row", (1, NQO), bf16, nc.sync)
            xf_r = load2("xf_r", (P, NJF, D), bf16, nc.sync)
            wa_bc = load2("wa_bc", (P, D), bf16, nc.sync)
            mask_col = load2("mask_col", (P, NJF, 1), bf16, nc.sync)
            wc["sup"] = load2("wc8_sup", (P, 8, D), f8e4, nc.sync,
                              pool=wcp, tag="wc", name_="wc_sup")
            wc["rep"] = load2("wc8_rep", (P, 8, D), f8e4, nc.sync,
                              pool=wcp, tag="wc", name_="wc_rep")
            xo_r = load2("xo_r", (P, NJO, D), bf16, nc.sync)
            w1 = load2("w1", (P, NKF, D), bf16, nc.sync, pool=wfuse,
                       tag="w1")
            w2 = load2("w2", (P, 7, D + 1), bf16, nc.sync, pool=wfuse,
                       tag="w2")
            if not sz.trivial_ln:
                lng = load2("lng", (BPC, D), fp32, nc.sync)
                lnb = load2("lnb", (BPC, D), fp32, nc.sync)
            # ACT table preloads (Exp, Tanh) after the scalar-ring issues
            nc.scalar.activation(junk1[:], zbias[0:1, :], AF.Exp,
                                 bias=zbias[0:1, :])
            nc.scalar.activation(junk1[:], zbias[0:1, :], AF.Tanh,
                                 bias=zbias[0:1, :])

            # ---- persistent small tiles ----
            anom_col = stats.tile([P, NJF, 2], fp32, tag="anom_col")
            ac_rows = stats.tile([1, NQF], bf16, tag="ac_rows")
            junk = stats.tile([P, D], fp32, tag="junk")
            e_t = stats.tile([P, NJF, 1], bf16, tag="e")
            eg = stats.tile([P, NJF, 1], bf16, tag="eg")
            rsg_row = stats.tile([1, BPC], bf16, tag="rsg_row")
            rsg_f32 = stats.tile([1, BPC], fp32, tag="rsg_f32")
            rsg_bc = stats.tile([P, BPC], fp32, tag="rsg_bc")
            Zs = {t: stats.tile([P, NJF, 1], fp32, tag=f"Z_{t}", name=f"Z_{t}")
                  for t in ("sup", "rep")}
            tanh_c = [stats.tile([P, NFC[i], 256], bf16, tag=f"tanh{i}",
                                 name=f"tanh{i}") for i in range(BPC)]
            G_all = stats.tile([P, NJF, BPC], bf16, tag="G_all")
            U_all = stats.tile([P, NJO, 2 * BPC], bf16, tag="U_all")
            F_rows = stats.tile([64, D], bf16, tag="F_rows")
            fusedT = stats.tile([P, 6, 64], bf16, tag="fusedT")
            hT = stats.tile([P, 7, BPC], bf16, tag="hT")
            h_sb = stats.tile([P, 256], bf16, tag="h_sb")

            def proj_chunk_fns(t):
                GT = gtp.tile([P, 6, NQF16], f8e4, tag="GT", name=f"GT_{t}")
                w_ = wc[t]
                fns = []
                for m in range(6):
                    for c0, cn in _chunks(NQF):
                        def fn(m=m, c0=c0, cn=cn, ci=len(fns)):
                            ps = pp_big.tile([P, cn], fp32, tag="big",
                                             name=f"ps_p{t}{m}c{c0}")
                            for kp in range(4):
                                nc.tensor.matmul(
                                    ps[:],
                                    w_[:, 2 * kp:2 * kp + 2,
                                       m * P:(m + 1) * P],
                                    xfT8[:, 2 * kp:2 * kp + 2, c0:c0 + cn],
                                    perf_mode=DR,
                                    start=(kp == 0), stop=(kp == 3))
                            if t == "con" and ci % 2 == 0:
                                nc.scalar.copy(GT[:, m, c0:c0 + cn], ps[:])
                            else:
                                nc.vector.tensor_copy(GT[:, m, c0:c0 + cn],
                                                      ps[:])
                        fns.append(fn)
                return GT, fns

            def emit_proj(t):
                GT, fns = proj_chunk_fns(t)
                for fn in fns:
                    fn()
                return GT

            def interleave(a_fns, b_fns):
                # emit a (score chunks) paced against b (next projection)
                bi = 0
                for k, fa in enumerate(a_fns):
                    fa()
                    tgt = (k + 1) * len(b_fns) // len(a_fns)
                    while bi < tgt:
                        b_fns[bi]()
                        bi += 1
                while bi < len(b_fns):
                    b_fns[bi]()
                    bi += 1

            def emit_ccol():
                # con's k-bias dot as an extra fp8 projection column
                for c0, cn in _chunks(NQF):
                    ps = pp_big.tile([1, cn], fp32, tag="big",
                                     name=f"ps_ac{c0}")
                    for kp in range(4):
                        nc.tensor.matmul(
                            ps[:],
                            wc["con"][:, 2 * kp:2 * kp + 2, 768:769],
                            xfT8[:, 2 * kp:2 * kp + 2, c0:c0 + cn],
                            perf_mode=DR, start=(kp == 0), stop=(kp == 3))
                    nc.scalar.copy(ac_rows[:, c0:c0 + cn], ps[:])
                nc.vector.memset(anom_col[:], NEGM)
                for i in range(BPC):
                    for rc in range(NFC[i]):
                        rows = min(P, fhat[i] - P * rc)
                        ps_t2 = pp_sm.tile([P, 1], bf16, tag="sm",
                                           name=f"ps_ac{i}{rc}")
                        nc.tensor.transpose(
                            ps_t2[0:rows, :],
                            ac_rows[:, FOFF[i] + P * rc:
                                    FOFF[i] + P * rc + rows],
                            eye[0:1, 0:1])
                        nc.vector.tensor_scalar_mul(
                            anom_col[:rows, JF[i] + rc, 1:2],
                            ps_t2[0:rows, :], 1.0 / SC)

            def emit_anom_dots():
                # anomaly dots in bf16 on DVE (one fused op per row tile);
                # mask_col then adds b_anom and the -30000 row padding mask
                for j in range(NJF):
                    nc.vector.scalar_tensor_tensor(
                        junk[:], xf_r[:, j, :], 1.0, wa_bc[:],
                        op0=ALU.mult, op1=ALU.mult,
                        accum_out=anom_col[:, j, 0:1])
                nc.vector.tensor_add(anom_col[:, :, 0:1],
                                     anom_col[:, :, 0:1], mask_col[:])

            def emit_gate():
                nc.scalar.activation(e_t[:], anom_col[:, :, 0:1], AF.Exp,
                                     bias=zbias[:])
                ps_sg = pp_sm.tile([1, BPC], fp32, tag="sm", name="ps_sg")
                for i in range(BPC):
                    for rc in range(NFC[i]):
                        nc.tensor.matmul(ps_sg[:, i:i + 1], ones_col[:],
                                         e_t[:, JF[i] + rc, :],
                                         start=(rc == 0),
                                         stop=(rc == NFC[i] - 1))
                nc.vector.reciprocal(rsg_f32[:], ps_sg[:])
                nc.vector.tensor_copy(rsg_row[:], rsg_f32[:])
                ps_rb = pp_sm.tile([P, BPC], fp32, tag="sm", name="ps_rb")
                nc.tensor.matmul(ps_rb[:], ones_row[0:1, 0:P], rsg_row[:])
                nc.vector.tensor_copy(rsg_bc[:], ps_rb[:])
                for i in range(BPC):
                    for rc in range(NFC[i]):
                        nc.vector.tensor_mul(eg[:, JF[i] + rc, :],
                                             e_t[:, JF[i] + rc, :],
                                             rsg_bc[:, i:i + 1])
                nc.vector.memset(G_all[:], 0.0)
                nc.vector.memset(U_all[:], 0.0)
                for i in range(BPC):
                    for rc in range(NFC[i]):
                        nc.vector.tensor_copy(G_all[:, JF[i] + rc, i:i + 1],
                                              eg[:, JF[i] + rc, :])

            def emit_scores(t, GT, i, rc):
                rows = min(P, fhat[i] - P * rc)
                o = ohat[i]
                ps_s = pp_s.tile([P, 256], fp32, tag="s", name=f"ps_s{t}{i}{rc}")
                for g in range(3):
                    nc.tensor.matmul(
                        ps_s[:rows, :o],
                        GT[:, 2 * g:2 * g + 2, FOFF[i] + P * rc:
                           FOFF[i] + P * rc + rows],
                        xoT8[:, 2 * g:2 * g + 2, P * JO[i]:P * JO[i] + o],
                        perf_mode=DR,
                        start=(g == 0), stop=(g == 2 and t == "con"))
                if t == "sup":
                    nc.tensor.matmul(
                        ps_s[:rows, :o], ones_row[0:1, 0:rows],
                        mo_row[0:1, P * JO[i]:P * JO[i] + o],
                        start=False, stop=True)
                elif t == "rep":
                    nc.tensor.matmul(
                        ps_s[:rows, :o], ones_row[0:1, 0:rows],
                        mo_row[0:1, P * JO[i]:P * JO[i] + o],
                        start=False, stop=False)
                    # += SC * tanh_c (identity matmul; eyesg = SC * I)
                    nc.tensor.matmul(
                        ps_s[:rows, :o], eyesg[:rows, :rows],
                        tanh_c[i][:rows, rc, :o],
                        start=False, stop=True)
                return ps_s, rows, o

            def emit_exp(t, i, rc, ps_s, rows, o, p_t):
                if t == "con":
                    nc.scalar.activation(tanh_c[i][:rows, rc, :o],
                                         ps_s[:rows, :o], AF.Tanh,
                                         scale=1.0 / SC,
                                         bias=anom_col[:rows, JF[i] + rc,
                                                       1:2])
                    return
                nc.scalar.activation(p_t[:rows, rc, :o], ps_s[:rows, :o],
                                     AF.Exp, scale=1.0 / SC,
                                     bias=zbias[:rows, :],
                                     accum_out=Zs[t][:rows, JF[i] + rc, :])

            def emit_u(t, i, p_t):
                tcol = 0 if t == "rep" else 1
                nfc = NFC[i]
                rz = soft.tile([P, nfc, 1], fp32, tag="rz", name=f"rz{t}{i}")
                nc.vector.reciprocal(rz[:], Zs[t][:, JF[i]:JF[i] + nfc, :])
                w_t = soft.tile([P, nfc, 1], bf16, tag="w", name=f"w{t}{i}")
                nc.vector.tensor_mul(w_t[:], eg[:, JF[i]:JF[i] + nfc, :],
                                     rz[:])
                ps_u = pp_sm.tile([P, NOC[i], 1], fp32, tag="sm",
                                  name=f"ps_u{t}{i}")
                for oc in range(NOC[i]):
                    cols = min(P, ohat[i] - P * oc)
                    for rc in range(nfc):
                        rows = min(P, fhat[i] - P * rc)
                        nc.tensor.matmul(
                            ps_u[:cols, oc, :],
                            p_t[:rows, rc, P * oc:P * oc + cols],
                            w_t[:rows, rc, :],
                            start=(rc == 0), stop=(rc == nfc - 1))
                for oc in range(NOC[i]):
                    cols = min(P, ohat[i] - P * oc)
                    nc.vector.tensor_copy(
                        U_all[:cols, JO[i] + oc, tcol * BPC + i:
                              tcol * BPC + i + 1],
                        ps_u[:cols, oc, :])

            # =========== emission ===========
            GT_con = emit_proj("con")
            emit_ccol()
            p_sup = [soft.tile([P, NFC[i], 256], f8e4, tag=f"Psup{i}",
                               name=f"Psup{i}") for i in range(BPC)]
            p_rep = [soft.tile([P, NFC[i], 256], f8e4, tag=f"Prep{i}",
                               name=f"Prep{i}") for i in range(BPC)]

            def score_fns(t, GT, p_ts):
                fns = []
                for i in range(BPC):
                    for rc in range(NFC[i]):
                        def fn(i=i, rc=rc):
                            ps_s, rows, o = emit_scores(t, GT, i, rc)
                            emit_exp(t, i, rc, ps_s, rows, o,
                                     p_ts[i] if p_ts else None)
                        fns.append(fn)
                return fns

            GT_sup, sup_fns = proj_chunk_fns("sup")
            interleave(score_fns("con", GT_con, None), sup_fns)
            emit_anom_dots()
            emit_gate()
            GT_rep, rep_fns = proj_chunk_fns("rep")
            interleave(score_fns("sup", GT_sup, p_sup), rep_fns)
            # interleave: rep scores for slot i, then sup u for slot i —
            # the u matmuls keep PE dense while ACT drains the rep exps
            for i in range(BPC):
                for rc in range(NFC[i]):
                    ps_s, rows, o = emit_scores("rep", GT_rep, i, rc)
                    emit_exp("rep", i, rc, ps_s, rows, o, p_rep[i])
                emit_u("sup", i, p_sup[i])
            # ---- tail: vecs -> fusedT -> h -> y -> layernorm ----
            # afv vec group first (G_all ready long ago), then per slot:
            # u_rep(i) followed by that slot's U-group j-tiles so the
            # option-vec accumulation streams behind the u matmuls
            nc.vector.memset(F_rows[:], 0.0)
            vchunks = _chunks(D)
            for ci, (c0, cn) in enumerate(vchunks):
                ps_v = pp_big.tile([BPC, cn], fp32, tag="big",
                                   name=f"ps_v{ci}")
                for j in range(NJF):
                    nc.tensor.matmul(ps_v[:], G_all[:, j, :],
                                     xf_r[:, j, c0:c0 + cn],
                                     start=(j == 0), stop=(j == NJF - 1))
                nc.scalar.copy(F_rows[32:32 + BPC, c0:c0 + cn], ps_v[:])
            ps_vus = [pp_big.tile([2 * BPC, cn], fp32, tag="big",
                                  name=f"ps_vu{ci}")
                      for ci, (c0, cn) in enumerate(vchunks)]
            for i in range(BPC):
                emit_u("rep", i, p_rep[i])
                for j in range(JO[i], JO[i + 1]):
                    for ci, (c0, cn) in enumerate(vchunks):
                        nc.tensor.matmul(ps_vus[ci][:], U_all[:, j, :],
                                         xo_r[:, j, c0:c0 + cn],
                                         start=(j == 0), stop=(j == NJO - 1))
            for ci, (c0, cn) in enumerate(vchunks):
                nc.vector.tensor_copy(F_rows[0:2 * BPC, c0:c0 + cn],
                                      ps_vus[ci][:])

            def filler(n=2):
                ps_fl = pp_s.tile([P, 512], fp32, tag="s", name="ps_fl")
                for r in range(n):
                    nc.tensor.matmul(ps_fl[:], scr[:, 0:P], scr[:],
                                     start=(r == 0), stop=(r == n - 1))
            filler()
            # [64, 768] -> fusedT[p, mj, row] via 6 PE transposes
            for mj in range(6):
                ps_f = pp_sm.tile([P, 64], bf16, tag="sm", name=f"ps_f{mj}")
                nc.tensor.transpose(ps_f[:], F_rows[:, mj * P:(mj + 1) * P],
                                    eye[0:64, 0:64])
                nc.vector.tensor_copy(fusedT[:, mj, :], ps_f[:])
            filler()

            # h = relu(W1^T fused + b1): 4 concurrent column groups
            ps_h = pp_s.tile([P, 256], fp32, tag="s", name="ps_h")
            FCOL = [32, 0, 4]        # fusedT col base for afv, wrv, wsv
            for t in range(3):
                for mj in range(6):
                    first = (t == 0 and mj == 0)
                    for e, (h0, hw) in enumerate(HW):
                        nc.tensor.matmul(
                            ps_h[32 * e:32 * e + BPC, 0:hw],
                            fusedT[:, mj, FCOL[t]:FCOL[t] + BPC],
                            w1[:, 6 * t + mj, h0:h0 + hw],
                            tile_position=(0, 32 * e),
                            start=first, stop=False)
            for e, (h0, hw) in enumerate(HW):
                nc.tensor.matmul(ps_h[32 * e:32 * e + BPC, 0:hw],
                                 ones_row[0:1, 0:BPC], w1[0:1, NKF - 1,
                                                          h0:h0 + hw],
                                 tile_position=(0, 32 * e),
                                 start=False, stop=True)
            for e, (h0, hw) in enumerate(HW):
                nc.scalar.activation(h_sb[32 * e:32 * e + BPC, 0:hw],
                                     ps_h[32 * e:32 * e + BPC, 0:hw],
                                     AF.Relu, bias=zbias[32 * e:32 * e + BPC, :])
            # Sqrt LUT preload; nothing between here and its use loads
            # a different ACT table
            nc.scalar.activation(junk1[:], eps_t[0:1, :], AF.Sqrt,
                                 bias=eps_t[0:1, :])
            # hT via PE transposes (all pieces 128-wide, base 0)
            nc.vector.memset(hT[:], 0.0)
            nc.vector.memset(hT[0:1, 6, :], 1.0)
            filler()
            pieces = [(0, 0, 0), (0, 128, 1), (1, 0, 2), (1, 128, 3),
                      (2, 0, 4), (3, 0, 5)]
            for e, l0, kt in pieces:
                ps_t = pp_sm.tile([P, BPC], bf16, tag="sm", name=f"ps_t{kt}")
                nc.tensor.transpose(ps_t[:], h_sb[32 * e:32 * e + BPC,
                                                  l0:l0 + P],
                                    eye[32 * e:32 * e + BPC, 64:64 + BPC],
                                    tile_position=(32 * e, 0))
                nc.vector.tensor_copy(hT[:, kt, :], ps_t[:])

            # y = h W2 + b2, with mean column 768
            ps_ys = []
            for ci, (c0, cn) in enumerate([(0, 512), (512, 257)]):
                ps_y = pp_big.tile([BPC, cn], fp32, tag="big",
                                   name=f"ps_y{ci}")
                ps_ys.append(ps_y)
                for kt in range(7):
                    nc.tensor.matmul(ps_y[:], hT[:, kt, :],
                                     w2[:, kt, c0:c0 + cn],
                                     start=(kt == 0), stop=(kt == 6))

            mu = stats.tile([BPC, 1], fp32, tag="mu")
            nc.vector.tensor_copy(mu[:], ps_ys[1][:, 256:257])
            xc = stats.tile([BPC, D], fp32, tag="xc")
            nc.vector.tensor_scalar(xc[:, 0:512], ps_ys[0][:], mu[:], None,
                                    op0=ALU.subtract)
            nc.vector.tensor_scalar(xc[:, 512:D], ps_ys[1][:, 0:256], mu[:],
                                    None, op0=ALU.subtract)
            var = stats.tile([BPC, 1], fp32, tag="var")
            junk2 = stats.tile([BPC, D], fp32, tag="junk2")
            nc.vector.scalar_tensor_tensor(junk2[:], xc[:], 1.0, xc[:],
                                           op0=ALU.mult, op1=ALU.mult,
                                           accum_out=var[:])
            sd = stats.tile([BPC, 1], fp32, tag="sd")
            nc.scalar.activation(sd[:], var[:], AF.Sqrt, scale=1.0 / D,
                                 bias=eps_t[0:BPC, :])
            rstd = stats.tile([BPC, 1], fp32, tag="rstd")
            nc.vector.reciprocal(rstd[:], sd[:])
            yt = stats.tile([BPC, D], fp32, tag="yt")
            nc.scalar.activation(yt[:], xc[:], AF.Identity, scale=rstd[:])
            if not sz.trivial_ln:
                nc.vector.tensor_mul(yt[:], yt[:], lng[:])
                nc.vector.tensor_add(yt[:], yt[:], lnb[:])
            nc.sync.dma_start(out=y_out[:], in_=yt[:])

    nc.compile()
    return nc


def _q8(a):
    return np.clip(a, -240.0, 240.0).astype(F8)


def _prep_inputs(x, x_ids, pad_idx, sep_idx, weights):
    false_mask, option_mask = _segment_masks(x_ids, pad_idx, sep_idx)
    f_cnt = false_mask.sum(1)
    o_cnt = option_mask.sum(1)
    order = np.argsort(-f_cnt, kind="stable")
    # slot i, core c <- order[8i + c]
    fhat = [int(max(f_cnt[order[NCORES * i:NCORES * (i + 1)]]))
            for i in range(BPC)]
    ohat = [int(max(o_cnt[order[NCORES * i:NCORES * (i + 1)]]))
            for i in range(BPC)]

    (W_anom, b_anom, Wq, bq, Wk, bk, W_fuse1, b_fuse1,
     W_fuse2, b_fuse2, ln_g, ln_b) = weights
    trivial = bool(np.all(ln_g == 1.0) and np.all(ln_b == 0.0))
    sz = Sizes(fhat, ohat, trivial)

    # ---- shared (weight) arrays ----
    shared = {}
    for t in ("sup", "con", "rep"):
        cq = np.concatenate([Wq[t], bq[t][None, :]], axis=0)  # [769, 768]
        ck = np.concatenate([Wk[t], bk[t][None, :]], axis=0)
        C = (cq @ ck.T) * INV                                  # [769, 769]
        wd = 784 if t == "con" else D
        w8 = np.zeros((8 * P, wd), np.float32)
        w8[:D + 1, :D] = C[:, :D] * SC
        if t == "con":
            # extra projection column: con's k-bias dot
            w8[:D + 1, D] = C[:, D] * SC
        shared[f"wc8_{t}"] = np.ascontiguousarray(
            _q8(w8).reshape(8, P, wd).transpose(1, 0, 2))
    a1 = np.zeros((NKF * P, D), np.float32)
    a1[:3 * D] = W_fuse1
    a1[3 * D] = b_fuse1
    shared["w1"] = np.ascontiguousarray(
        a1.astype(BF16).reshape(NKF, P, D).transpose(1, 0, 2))
    a2 = np.zeros((7 * P, D + 1), np.float32)
    a2[:D, :D] = W_fuse2
    a2[D, :D] = b_fuse2
    a2[:D, D] = W_fuse2.mean(axis=1)
    a2[D, D] = b_fuse2.mean()
    shared["w2"] = np.ascontiguousarray(
        a2.astype(BF16).reshape(7, P, D + 1).transpose(1, 0, 2))
    shared["wa_bc"] = np.ascontiguousarray(
        np.broadcast_to(W_anom[:, 0][None, :], (P, D))).astype(BF16)
    eye_all = np.zeros((P, 68), np.float32)
    eye_all[:64, :64] = np.eye(64)
    for e in range(4):
        eye_all[32 * e:32 * e + BPC, 64:64 + BPC] = np.eye(BPC)
    shared["eye"] = eye_all.astype(BF16)
    shared["eyesg"] = (np.eye(P) * SC).astype(BF16)
    if not trivial:
        shared["lng"] = np.ascontiguousarray(
            np.broadcast_to(ln_g[None, :], (BPC, D)).astype(np.float32))
        shared["lnb"] = np.ascontiguousarray(
            np.broadcast_to(ln_b[None, :], (BPC, D)).astype(np.float32))

    in_maps = []
    for c in range(NCORES):
        xfT_f = np.zeros((8 * P, sz.NQF16), F8)
        xoT_f = np.zeros((6 * P, sz.NQO), F8)
        xf_r = np.zeros((sz.NJF * P, D), np.float32)
        xo_r = np.zeros((sz.NJO * P, D), np.float32)
        mo = np.zeros((1, sz.NQO), np.float32)
        mcol = np.full((sz.NJF * P,), NEGM, np.float32)
        for i in range(BPC):
            gb = order[NCORES * i + c]
            f_idx = np.where(false_mask[gb])[0]
            o_idx = np.where(option_mask[gb])[0]
            nf, no = len(f_idx), len(o_idx)
            xf = x[gb, f_idx]                       # [nf, 768]
            xo = x[gb, o_idx]
            # packed feature-major false rows (+ones row 768)
            q0 = sz.FOFF[i]
            xfT_f[:D, q0:q0 + nf] = _q8(xf.T)
            xfT_f[D, q0:q0 + nf] = np.float32(1.0)
            # 128-padded feature-major options
            o0 = P * sz.JO[i]
            xoT_f[:D, o0:o0 + no] = _q8(xo.T)
            mo[0, o0 + no:o0 + ohat[i]] = NEGM * SC
            mcol[P * sz.JF[i]:P * sz.JF[i] + nf] = b_anom[0]
            # row-major tiles
            r0 = P * sz.JF[i]
            xf_r[r0:r0 + nf, :] = xf
            xo_r[o0:o0 + no, :] = xo

        m = dict(shared)
        m["xfT8"] = np.ascontiguousarray(
            xfT_f.reshape(8, P, sz.NQF16).transpose(1, 0, 2))
        m["xoT8"] = np.ascontiguousarray(
            xoT_f.reshape(6, P, sz.NQO).transpose(1, 0, 2))
        m["mo_row"] = mo.astype(BF16)
        m["mask_col"] = np.ascontiguousarray(
            mcol.reshape(sz.NJF, P, 1).transpose(1, 0, 2)).astype(BF16)
        m["xf_r"] = np.ascontiguousarray(
            xf_r.astype(BF16).reshape(sz.NJF, P, D).transpose(1, 0, 2))
        m["xo_r"] = np.ascontiguousarray(
            xo_r.astype(BF16).reshape(sz.NJO, P, D).transpose(1, 0, 2))
        in_maps.append(m)
    return sz, order, in_maps


_CACHE = {}
LAST_RESULTS = None


def kernel(x, x_ids, pad_idx, sep_idx,
           W_anom, b_anom,
           Wq_sup, bq_sup, Wk_sup, bk_sup,
           Wq_con, bq_con, Wk_con, bk_con,
           Wq_rep, bq_rep, Wk_rep, bk_rep,
           W_fuse1, b_fuse1, W_fuse2, b_fuse2,
           ln_g, ln_b):
    from concourse import bass_utils

    global LAST_RESULTS
    x = np.asarray(x, np.float32)
    x_ids = np.asarray(x_ids)
    pad_idx = int(np.asarray(pad_idx))
    sep_idx = int(np.asarray(sep_idx))
    weights = (
        np.asarray(W_anom, np.float32), np.asarray(b_anom, np.float32),
        {"sup": np.asarray(Wq_sup, np.float32),
         "con": np.asarray(Wq_con, np.float32),
         "rep": np.asarray(Wq_rep, np.float32)},
        {"sup": np.asarray(bq_sup, np.float32),
         "con": np.asarray(bq_con, np.float32),
         "rep": np.asarray(bq_rep, np.float32)},
        {"sup": np.asarray(Wk_sup, np.float32),
         "con": np.asarray(Wk_con, np.float32),
         "rep": np.asarray(Wk_rep, np.float32)},
        {"sup": np.asarray(bk_sup, np.float32),
         "con": np.asarray(bk_con, np.float32),
         "rep": np.asarray(bk_rep, np.float32)},
        np.asarray(W_fuse1, np.float32), np.asarray(b_fuse1, np.float32),
        np.asarray(W_fuse2, np.float32), np.asarray(b_fuse2, np.float32),
        np.asarray(ln_g, np.float32), np.asarray(ln_b, np.float32),
    )

    sz, order, in_maps = _prep_inputs(x, x_ids, pad_idx, sep_idx, weights)
    key = sz.key()
    if key not in _CACHE:
        _CACHE[key] = _build_program(sz)
    nc = _CACHE[key]
    last_err = None
    for attempt in range(3):
        try:
            res = bass_utils.run_bass_kernel_spmd(
                nc, in_maps, list(range(NCORES)))
            break
        except Exception as err:  # transient device-unrecoverable states
            last_err = err
            import time
            time.sleep(5 * (attempt + 1))
            try:
                import jax.extend
                jax.extend.backend.clear_backends()
            except Exception:
                pass
    else:
        raise last_err
    LAST_RESULTS = res
    out = np.zeros((B, D), np.float32)
    for c in range(NCORES):
        for i in range(BPC):
            out[order[NCORES * i + c]] = res.results[c]["y"][i]
    return out
